# revision 16
# baseline (speedup 1.0000x reference)
"""Multi-head attention Trainium2 Bass kernel (bf16, DP4 x TP2, pipelined).

Problem: B=4, S=2048, H=16, DH=64, D=1024, fp32 inputs.
  q/k/v = hidden @ W{q,k,v}.T + b; scores = q k^T / 8; probs = softmax;
  ctx = probs v; out = ctx @ Wo.T + bo.

Sharding: batch data-parallel x head tensor-parallel. Core c owns batch
c//2 and heads 8*(c%2)..+8 (feature slice of 512). Host sums the 2
partial output projections per batch and adds bo.

Host-side prep: inputs cast to bf16 and laid out exactly as the SBUF
tiles want them (h^T and all W^T pre-transposed, e-chunked), so the
device does no PE transposes at all.

Device schedule (single Act engine paces the exps; everything else is
arranged to hide under it):
  phase 0: DMAs (wq, wk first), Q^T/K^T projection for head-pair 0
           (e-outer, 4 open PSUM groups -> LDWEIGHTS shared across 4
           512-col matmuls).
  attention units (hp, qs) for head-pair hp, 512-wide q-span qs:
    per kt-chunk c: packed scores pair (heads 2hp/2hp+1 at PE row-tiles
    (0,0)/(64,0), concurrent), one exp [128,2,512] (1024 elem/lane),
    ctx pair into psc[65,2,512]. Fillers sprinkled one per chunk:
    V projection units (hp0/qs0, per-kt-chunk, feeding ctx of the same
    chunk), Q/K projections for hp1-3, output projection for earlier
    q-spans. Denominator: row 64 of psc (ones column in Vaug), hopped
    to partition 0 via SBUF DMA, partition-broadcast, fast reciprocal.
"""
import numpy as np

import concourse.bass as bass
import concourse.tile as tile
from concourse import bacc, mybir
from concourse import bass_utils

F32 = mybir.dt.float32
BF16 = mybir.dt.bfloat16
FP8 = mybir.dt.float8e4
DR = mybir.MatmulPerfMode.DoubleRow
EXP = mybir.ActivationFunctionType.Exp
ADD = mybir.AluOpType.add
MULT = mybir.AluOpType.mult

B = 4
S = 2048
D = 1024
NCORES = 8
P = 128          # partitions
EC = D // P      # 8 e-chunks
KC = S // P      # 16 kt chunks
F = 512          # per-core feature slice (8 heads x 64)
FC = F // P      # 4 f-chunks == head-pairs
NHP = 4          # head pairs per core
QS = 512         # q span per attention unit
NQS = S // QS    # 4 q spans


def build_nc():
    nc = bacc.Bacc("TRN2", target_bir_lowering=False, debug=False,
                   enable_asserts=True, num_devices=NCORES)

    hT = nc.dram_tensor("ht", [P, EC * S], BF16, kind="ExternalInput").ap()
    wq = nc.dram_tensor("wq", [P, EC * F], BF16, kind="ExternalInput").ap()
    wk = nc.dram_tensor("wk", [P, EC * F], BF16, kind="ExternalInput").ap()
    wv = nc.dram_tensor("wv", [P, EC * F], BF16, kind="ExternalInput").ap()
    wo = nc.dram_tensor("wo", [P, FC * D], BF16, kind="ExternalInput").ap()
    bq = nc.dram_tensor("bq", [P, FC], F32, kind="ExternalInput").ap()
    bk = nc.dram_tensor("bk", [P, FC], F32, kind="ExternalInput").ap()
    bv = nc.dram_tensor("bv", [1, F], F32, kind="ExternalInput").ap()
    out = nc.dram_tensor("out", [S, D], F32, kind="ExternalOutput").ap()

    with tile.TileContext(nc) as tc:
        with (
            tc.tile_pool(name="const", bufs=1) as cpool,
            tc.tile_pool(name="wts", bufs=1) as wpool,
            tc.tile_pool(name="big", bufs=1) as bigp,
            tc.tile_pool(name="probs", bufs=3) as probsp,
            tc.tile_pool(name="csbp", bufs=2) as csbp,
            tc.tile_pool(name="recipp", bufs=2) as recipp,
            tc.tile_pool(name="recipbp", bufs=2) as recipbp,
            tc.tile_pool(name="ostage", bufs=2) as ostage,
            tc.tile_pool(name="pgrp", bufs=2, space="PSUM") as pgrp,
        ):
            # ---- input DMAs, weights first so PE can start ASAP ----
            wqT = wpool.tile([P, EC, F], BF16, tag="wqT")
            wkT = wpool.tile([P, EC, F], BF16, tag="wkT")
            hts = []
            for e in range(EC):
                ht_e = bigp.tile([P, S], BF16, tag=f"ht{e}", name=f"ht{e}")
                hts.append(ht_e)
            for e in range(EC):
                nc.sync.dma_start(wqT[:, e, :], wq[:, e * F:(e + 1) * F])
                nc.sync.dma_start(wkT[:, e, :], wk[:, e * F:(e + 1) * F])
                nc.sync.dma_start(hts[e][:], hT[:, e * S:(e + 1) * S])
            wvT = wpool.tile([P, EC, F], BF16, tag="wvT")
            woT = wpool.tile([P, FC, D], BF16, tag="woT")
            nc.sync.dma_start(wvT[:], wv.rearrange("p (e f) -> p e f", e=EC))
            nc.sync.dma_start(woT[:], wo.rearrange("p (c d) -> p c d", c=FC))

            ones32 = cpool.tile([P, 1], F32)
            nc.gpsimd.memset(ones32[:], 1.0)
            bq_t = cpool.tile([P, FC], F32, tag="bq")
            bk_t = cpool.tile([P, FC], F32, tag="bk")
            bv_row = cpool.tile([1, F], F32, tag="bvr")
            nc.sync.dma_start(bq_t[:], bq)
            nc.sync.dma_start(bk_t[:], bk)
            nc.sync.dma_start(bv_row[:], bv)
            bv_b = cpool.tile([P, F], F32, tag="bvb")
            nc.gpsimd.partition_broadcast(bv_b[:], bv_row[0:1, :])

            qT = bigp.tile([P, FC, S], BF16, tag="qT")
            kT = bigp.tile([P, FC, S], BF16, tag="kT")
            # vaug[tok, chunk, head, 0:64] = V, [..., 64] = 1 (denom)
            vaug = bigp.tile([P, KC, 2 * NHP, 65], BF16, tag="vaug")
            ctxn = bigp.tile([P, FC, S], BF16, tag="ctxn")

            nc.vector.tensor_copy(
                vaug[:, :, :, 64:65],
                ones32[:, None, None, :].to_broadcast((P, KC, 2 * NHP, 1)),
            )

            # ---- Q/K projection for one head-pair (= f-chunk) ----
            def qk_proj_full(fc, pool):
                """e-outer, 4 open 512-col groups: LDWEIGHTS shared x4."""
                for wT, b_t, is_q in ((wqT, bq_t, True), (wkT, bk_t, False)):
                    grps = []
                    for gi in range(4):
                        g = pool.tile([P, 512], F32, tag="proj", name=f"g{gi}")
                        grps.append(g)
                    for e in range(EC):
                        for tt in range(4):
                            nc.tensor.matmul(
                                grps[tt][:], wT[:, e, bass.ts(fc, P)],
                                hts[e][:, bass.ts(tt, 512)],
                                start=(e == 0), stop=(e == EC - 1))
                    for tt in range(4):
                        if is_q:
                            nc.vector.tensor_scalar(
                                qT[:, fc, bass.ts(tt, 512)], grps[tt][:],
                                bq_t[:, fc:fc + 1], 0.125, ADD, MULT)
                        else:
                            nc.vector.tensor_scalar_add(
                                kT[:, fc, bass.ts(tt, 512)], grps[tt][:],
                                bk_t[:, fc:fc + 1])

            # ---- filler generators: yield once per emitted micro-step ----
            def qk_proj_steps(fc):
                """Q/K proj via pgrp (2 open groups, LDWEIGHTS shared x2)."""
                for wT, b_t, is_q in ((wqT, bq_t, True), (wkT, bk_t, False)):
                    for tp in range(2):          # tt pairs (0,1), (2,3)
                        g0 = pgrp.tile([P, 512], F32, tag="fill")
                        g1 = pgrp.tile([P, 512], F32, tag="fill")
                        for e in range(EC):
                            nc.tensor.matmul(
                                g0[:], wT[:, e, bass.ts(fc, P)],
                                hts[e][:, bass.ts(2 * tp, 512)],
                                start=(e == 0), stop=(e == EC - 1))
                            nc.tensor.matmul(
                                g1[:], wT[:, e, bass.ts(fc, P)],
                                hts[e][:, bass.ts(2 * tp + 1, 512)],
                                start=(e == 0), stop=(e == EC - 1))
                            yield
                        for i, g in enumerate((g0, g1)):
                            tt = 2 * tp + i
                            if is_q:
                                nc.vector.tensor_scalar(
                                    qT[:, fc, bass.ts(tt, 512)], g[:],
                                    bq_t[:, fc:fc + 1], 0.125, ADD, MULT)
                            else:
                                nc.vector.tensor_scalar_add(
                                    kT[:, fc, bass.ts(tt, 512)], g[:],
                                    bk_t[:, fc:fc + 1])
                        yield

            def v_unit(tc_i):
                """V for kt-chunk tc_i: [tok, f] via ht-stationary matmuls."""
                pv = pgrp.tile([P, F], F32, tag="fill")
                for e in range(EC):
                    nc.tensor.matmul(
                        pv[:], hts[e][:, bass.ts(tc_i, P)], wvT[:, e, :],
                        start=(e == 0), stop=(e == EC - 1))
                nc.vector.tensor_tensor(
                    vaug[:, tc_i, :, 0:64],
                    pv[:].rearrange("p (h f) -> p h f", h=2 * NHP),
                    bv_b[:].rearrange("p (h f) -> p h f", h=2 * NHP),
                    ADD)

            def outproj_steps(qs):
                """Output projection for q-span qs (4 token chunks)."""
                for st in range(QS // P):
                    t0 = qs * QS + st * P
                    po0 = pgrp.tile([P, 512], F32, tag="fill")
                    po1 = pgrp.tile([P, 512], F32, tag="fill")
                    for fc in range(FC):
                        nc.tensor.matmul(po0[:], ctxn[:, fc, t0:t0 + P],
                                         woT[:, fc, 0:512],
                                         start=(fc == 0), stop=(fc == FC - 1))
                        nc.tensor.matmul(po1[:], ctxn[:, fc, t0:t0 + P],
                                         woT[:, fc, 512:1024],
                                         start=(fc == 0), stop=(fc == FC - 1))
                        yield
                    ot = ostage.tile([P, D], F32)
                    nc.vector.tensor_copy(ot[:, 0:512], po0[:])
                    nc.vector.tensor_copy(ot[:, 512:1024], po1[:])
                    nc.sync.dma_start(out[t0:t0 + P, :], ot[:])
                    yield

            # ---- phase 0: Q/K for head-pair 0 (dedicated 4-buf pool) ----
            with tc.tile_pool(name="ps_qk0", bufs=4, space="PSUM") as projp:
                qk_proj_full(0, projp)

            # ---- attention ----
            with (
                tc.tile_pool(name="ps_scores", bufs=2, space="PSUM") as ps_s,
                tc.tile_pool(name="ps_ctx", bufs=1, space="PSUM") as ps_c,
            ):
                # global filler queue; generators appended once their data
                # deps are resolvable without blocking the in-order PE
                filler_q = []

                def filler_step():
                    while filler_q:
                        try:
                            next(filler_q[0])
                            return
                        except StopIteration:
                            filler_q.pop(0)

                # new fillers become legal at these (hp, qs) points:
                # Q/K for head-pair k needs only inputs; outproj(qs) needs
                # ctxn of ALL head-pairs at qs -> legal from (3, qs+1) on.
                additions = {
                    (0, 1): [qk_proj_steps(1)],
                    (1, 0): [qk_proj_steps(2)],
                    (2, 0): [qk_proj_steps(3)],
                    (3, 1): [outproj_steps(0)],
                    (3, 2): [outproj_steps(1)],
                    (3, 3): [outproj_steps(2)],
                }

                for hp in range(NHP):
                    for qs in range(NQS):
                        filler_q.extend(additions.get((hp, qs), []))
                        q0 = qs * QS
                        psc = ps_c.tile([65, 2, QS], F32, tag="ctx")
                        prev = None
                        for c in range(KC):
                            if hp == 0 and qs == 0:
                                v_unit(c)
                            pss = ps_s.tile([P, 2, QS], F32, tag="scores")
                            # packed pair: heads 2hp (rows 0:64) and 2hp+1
                            # (rows 64:128) run concurrently as PE row-tiles
                            nc.tensor.matmul(
                                pss[:, 0, :],
                                kT[0:64, hp, bass.ts(c, P)],
                                qT[0:64, hp, q0:q0 + QS],
                                start=True, stop=True)
                            nc.tensor.matmul(
                                pss[:, 1, :],
                                kT[64:128, hp, bass.ts(c, P)],
                                qT[64:128, hp, q0:q0 + QS],
                                start=True, stop=True)
                            pr = probsp.tile([P, 2, QS], BF16)
                            nc.scalar.activation(pr[:], pss[:], EXP)
                            if prev is not None:
                                pv_, cc = prev
                                for i in range(2):
                                    nc.tensor.matmul(
                                        psc[:, i, :],
                                        vaug[:, cc, 2 * hp + i, :],
                                        pv_[:, i, :],
                                        start=(cc == 0), stop=False)
                            prev = (pr, c)
                            if not (hp == 0 and qs == 0):
                                filler_step()
                                filler_step()
                        pv_, cc = prev
                        for i in range(2):
                            nc.tensor.matmul(
                                psc[:, i, :], vaug[:, cc, 2 * hp + i, :],
                                pv_[:, i, :], start=False, stop=True)

                        # normalize: den row 64 -> recip broadcast -> mult
                        csb = csbp.tile([65, 2, QS], F32)
                        nc.vector.tensor_copy(csb[:], psc[:])
                        rc = recipp.tile([1, 2 * QS], F32)
                        nc.gpsimd.dma_start(
                            rc[0:1, :],
                            csb[64:65, :, :].rearrange("p a b -> p (a b)"))
                        rb = recipbp.tile([64, 2 * QS], F32)
                        nc.gpsimd.partition_broadcast(rb[:], rc[0:1, :])
                        nc.vector.reciprocal_approx_fast(rb[:], rb[:])
                        nc.vector.tensor_tensor(
                            ctxn[0:64, hp, q0:q0 + QS],
                            csb[0:64, 0, :], rb[:, 0:QS], MULT)
                        nc.vector.tensor_tensor(
                            ctxn[64:128, hp, q0:q0 + QS],
                            csb[0:64, 1, :], rb[:, QS:2 * QS], MULT)

                # drain remaining fillers (tail of outproj qs0-2)
                while filler_q:
                    try:
                        next(filler_q[0])
                    except StopIteration:
                        filler_q.pop(0)
                # final q-span output projection
                for _ in outproj_steps(3):
                    pass

    nc.compile()
    return nc


_NC_CACHE = None


def build_in_maps(hidden_states, Wq, bq, Wk, bk, Wv, bv, Wo):
    hid = np.asarray(hidden_states, np.float32)
    Wq = np.asarray(Wq, np.float32)
    Wk = np.asarray(Wk, np.float32)
    Wv = np.asarray(Wv, np.float32)
    Wo = np.asarray(Wo, np.float32)

    in_maps = []
    for c in range(NCORES):
        b = c // 2
        fs = (c % 2) * F
        sl = slice(fs, fs + F)
        hTb = hid[b].T  # [D, S]
        hTb = hTb.reshape(EC, P, S).transpose(1, 0, 2).reshape(P, EC * S)
        wqT = Wq[sl].T.reshape(EC, P, F).transpose(1, 0, 2).reshape(P, -1)
        wkT = Wk[sl].T.reshape(EC, P, F).transpose(1, 0, 2).reshape(P, -1)
        wvT = Wv[sl].T.reshape(EC, P, F).transpose(1, 0, 2).reshape(P, -1)
        # Wo[:, sl] is [D, F]; transpose -> [F, D] = [f, fo], f-chunked
        woT = Wo[:, sl].T.reshape(FC, P, D).transpose(1, 0, 2).reshape(P, -1)
        in_maps.append({
            "ht": to_bf16(hTb),
            "wq": to_bf16(wqT),
            "wk": to_bf16(wkT),
            "wv": to_bf16(wvT),
            "wo": to_bf16(woT),
            "bq": np.ascontiguousarray(
                np.asarray(bq, np.float32)[sl].reshape(FC, P).T),
            "bk": np.ascontiguousarray(
                np.asarray(bk, np.float32)[sl].reshape(FC, P).T),
            "bv": np.ascontiguousarray(
                np.asarray(bv, np.float32)[sl].reshape(1, F)),
        })
    return in_maps


def to_bf16(a):
    import ml_dtypes
    return np.ascontiguousarray(a.astype(ml_dtypes.bfloat16))


def kernel(hidden_states, Wq, bq, Wk, bk, Wv, bv, Wo, bo):
    global _NC_CACHE
    if _NC_CACHE is None:
        _NC_CACHE = build_nc()
    nc = _NC_CACHE

    in_maps = build_in_maps(hidden_states, Wq, bq, Wk, bk, Wv, bv, Wo)

    try:
        res = bass_utils.run_bass_kernel_spmd(nc, in_maps,
                                              core_ids=list(range(NCORES)))
    except Exception:
        # transient device flake: retry once
        res = bass_utils.run_bass_kernel_spmd(nc, in_maps,
                                              core_ids=list(range(NCORES)))
    bo = np.asarray(bo, dtype=np.float32)
    full = np.empty((B, S, D), dtype=np.float32)
    for b in range(B):
        full[b] = res.results[2 * b]["out"].astype(np.float32)
        full[b] += res.results[2 * b + 1]["out"]
        full[b] += bo
    return full
